# revision 2
# baseline (speedup 1.0000x reference)
"""DNC-style LSTM-with-memory-read kernel for 8 Trainium2 NeuronCores.

Math summary (derived from the reference):
  The torch-faithful [R,B,M]->[B,R*M] view means row b' of the new read
  vector is concat_k read[(4*b'+k) mod B]. Since read = h @ mem_sm.T and
  rv only enters the LSTM through W_ih's rv columns (W_rv), the rv
  contribution to the gates collapses to a "mix" term:
      gates[b'] += sum_k h[(4*u(b')+k) mod B] @ G_k,   u(b') = b' mod 256
  with G_k = mem_sm.T @ W_rv[:, k*M:(k+1)*M].T precomputed on host.
  The final fc layer is linear in h and read, and the output is a mean
  over time, so it reduces to a function of hsum = sum_t h_t — computed
  on host from each core's hsum shard.

Distribution (v3): batch sharded contiguously over 8 cores (128 rows
each), everything in transposed layout [units, batch] so h feeds the
next step's matmuls with no transposes. Each step AllGathers h (bf16,
32KB/core); each core DMA-reads the half of the gathered buffer its
parity needs (even cores need global batch cols [0,512), odd [512,1024))
and computes 16 mix matmuls with the gathered window as rhs. The
x-projection and W_hh matmuls are independent of the gathered data and
run during the collective. Gate pre-activations accumulate in 4
dedicated PSUM banks (one per gate, so start=True bank clears never
interact); ACT applies sigmoid/tanh with the per-unit bias fused.
"""

import sys

if '/opt/trn_rl_repo' not in sys.path:
    sys.path.insert(0, '/opt/trn_rl_repo')

import numpy as np

B, T, D_IN = 1024, 128, 256
H = 128
M = 128
W = 128
R = 4
OUT = 2
NCORES = 8
RL = B // NCORES  # 128 local rows per core

_PROGRAM_CACHE = {}


def build_program_v3(t_steps=T):
    """Batch-sharded recurrence with a per-step bf16 AllGather of h.

    Layouts (per core, T-layout = [feature/unit partitions, batch cols]):
      h, c:   [128, RL]
      gates:  4 PSUM banks [128, 512] (only cols 0:RL used) — one bank
              per gate so each gate's start=True clear is bank-private.
      hb:     h cast to bf16 with columns permuted k-major
              (hb[:, k*32+j] = h[:, 4j+k]) so the gathered window is
              consumed by the mix matmuls in 32-contiguous runs.
    """
    import concourse.bass as bass
    import concourse.bacc as bacc
    import concourse.mybir as mybir
    import concourse.tile as tile

    f32 = mybir.dt.float32
    bf16 = mybir.dt.bfloat16
    AF = mybir.ActivationFunctionType
    x_t_in = min(t_steps, T)

    nc = bacc.Bacc(
        "TRN2",
        target_bir_lowering=False,
        debug=False,
        enable_asserts=False,
        num_devices=NCORES,
    )

    xT = nc.dram_tensor("xT", [x_t_in, 128, 2, RL], f32, kind="ExternalInput")
    wxT = nc.dram_tensor("wxT", [128, 2, 512], f32, kind="ExternalInput")
    whhT = nc.dram_tensor("whhT", [128, 512], f32, kind="ExternalInput")
    gmat = nc.dram_tensor("gmat", [128, 4, 512], bf16, kind="ExternalInput")
    biasc = nc.dram_tensor("biasc", [128, 4], f32, kind="ExternalInput")
    bias1c = nc.dram_tensor("bias1c", [128, 4], f32, kind="ExternalInput")
    hsum_out = nc.dram_tensor("hsum_out", [128, RL], f32, kind="ExternalOutput")

    FN = {0: AF.Sigmoid, 1: AF.Sigmoid, 2: AF.Tanh, 3: AF.Sigmoid}
    ORDER = (2, 0, 1, 3)  # g, i, f, o: t2=i*g first, o last

    with tile.TileContext(nc) as tc:
        with (
            tc.tile_pool(name="const", bufs=1) as cpool,
            tc.tile_pool(name="xin", bufs=4) as xpool,
            tc.tile_pool(name="work", bufs=2) as wpool,
            tc.tile_pool(name="hw", bufs=3) as hpool,
            tc.tile_pool(name="psg", bufs=2, space="PSUM") as psg,
            tc.tile_pool(name="dram", bufs=3, space="DRAM") as dpool,
        ):
            wx_sb = cpool.tile([128, 2, 512], f32)
            nc.sync.dma_start(wx_sb[:], wxT[:])
            whh_sb = cpool.tile([128, 512], f32)
            nc.sync.dma_start(whh_sb[:], whhT[:])
            g_sb = cpool.tile([128, 4, 512], bf16)
            nc.sync.dma_start(g_sb[:], gmat[:])
            bb_sb = cpool.tile([128, 4], f32)
            nc.sync.dma_start(bb_sb[:], biasc[:])
            b1_sb = cpool.tile([128, 4], f32)
            nc.sync.dma_start(b1_sb[:], bias1c[:])
            hsum = cpool.tile([128, RL], f32)
            nc.vector.memset(hsum[:], 0.0)

            # which 4 rank-blocks of the gathered h this core's mix needs
            pid = nc.sync.partition_id()
            roff = nc.sync.compute_val((pid % 2) * 4)

            h_prev = None
            c_prev = None
            hwin = None

            for t in range(1, t_steps + 1):
                xt = xpool.tile([128, 2, RL], f32, tag="xt")
                nc.sync.dma_start(xt[:], xT[(t - 1) % x_t_in])

                # ---- gate pre-activations: 4 PSUM banks, one per gate.
                # x-projection + W_hh first (independent of the AllGather,
                # they run while the collective is in flight), mix last.
                pg = {}
                for g in ORDER:
                    gsl = slice(128 * g, 128 * (g + 1))
                    p = psg.tile([128, 512], f32, tag=f"pg{g}", name=f"pg{g}")
                    pg[g] = p
                    nc.tensor.matmul(
                        p[:, 0:RL], wx_sb[:, 0, gsl], xt[:, 0, :],
                        start=True, stop=False,
                    )
                    nc.tensor.matmul(
                        p[:, 0:RL], wx_sb[:, 1, gsl], xt[:, 1, :],
                        start=False, stop=(t == 1),
                    )
                if t >= 2:
                    for g in ORDER:
                        gsl = slice(128 * g, 128 * (g + 1))
                        nc.tensor.matmul(
                            pg[g][:, 0:RL], whh_sb[:, gsl], h_prev[:],
                            start=False, stop=False,
                        )
                    for g in ORDER:
                        gsl = slice(128 * g, 128 * (g + 1))
                        for k in range(4):
                            nc.tensor.matmul(
                                pg[g][:, 0:RL],
                                g_sb[:, k, gsl],
                                hwin[:, :, 32 * k:32 * (k + 1)],
                                start=False, stop=(k == 3),
                            )

                bias_t = b1_sb if t == 1 else bb_sb
                a = {}
                for g in ORDER:
                    a[g] = wpool.tile([128, RL], f32, tag=f"a{g}", name=f"a{g}")
                    nc.scalar.activation(
                        a[g][:], pg[g][:, 0:RL], FN[g], bias=bias_t[:, g:g + 1]
                    )

                t2 = wpool.tile([128, RL], f32, tag="t2")
                nc.vector.tensor_mul(t2[:], a[0][:], a[2][:])
                c_new = wpool.tile([128, RL], f32, tag="c")
                if t == 1:
                    nc.vector.tensor_copy(c_new[:], t2[:])
                else:
                    t1 = wpool.tile([128, RL], f32, tag="t1")
                    nc.vector.tensor_mul(t1[:], a[1][:], c_prev[:])
                    nc.vector.tensor_add(c_new[:], t1[:], t2[:])
                c_prev = c_new
                tch = wpool.tile([128, RL], f32, tag="tch")
                nc.scalar.activation(tch[:], c_new[:], AF.Tanh)
                h = wpool.tile([128, RL], f32, tag="h")
                nc.vector.tensor_mul(h[:], a[3][:], tch[:])
                nc.vector.tensor_add(hsum[:], hsum[:], h[:])
                h_prev = h

                if t < t_steps:
                    # bf16 cast + k-major column permute, then AllGather
                    hb = wpool.tile([128, 4, 32], bf16, tag="hb")
                    nc.vector.tensor_copy(
                        hb[:], h.rearrange("p (j k) -> p k j", k=4)
                    )
                    ag_in = dpool.tile([128, RL], bf16, tag="agin")
                    nc.sync.dma_start(
                        ag_in[:], hb.rearrange("p k j -> p (k j)")
                    )
                    ag_out = dpool.tile(
                        [NCORES * 128, RL], bf16, tag="agout",
                        addr_space="Shared",
                    )
                    nc.gpsimd.collective_compute(
                        "AllGather",
                        mybir.AluOpType.bypass,
                        replica_groups=[list(range(NCORES))],
                        ins=[ag_in[:]],
                        outs=[ag_out[:]],
                    )
                    hwin = hpool.tile([128, 4, RL], bf16, tag="hwin")
                    nc.sync.dma_start(
                        hwin[:],
                        ag_out.rearrange("(r p) j -> p r j", p=128)[
                            :, bass.ds(roff, 4), :
                        ],
                    )

            nc.sync.dma_start(hsum_out[:], hsum[:])

    nc.compile()
    return nc


def host_prep(inputs, t_steps=T, mode="v3"):
    """Host-side parameter folding + per-core input maps."""
    import ml_dtypes

    x = np.asarray(inputs["x"], dtype=np.float32)
    memory = np.asarray(inputs["memory"], dtype=np.float64)
    rv0 = np.asarray(inputs["read_vectors0"], dtype=np.float64)
    W_ih = np.asarray(inputs["W_ih"], dtype=np.float64)
    W_hh = np.asarray(inputs["W_hh"], dtype=np.float64)
    b_ih = np.asarray(inputs["b_ih"], dtype=np.float64)
    b_hh = np.asarray(inputs["b_hh"], dtype=np.float64)

    # softmax over memory slots (dim 0)
    mm = memory - memory.max(axis=0, keepdims=True)
    e = np.exp(mm)
    mem_sm = e / e.sum(axis=0, keepdims=True)  # [M, W]

    W_x = W_ih[:, :D_IN]          # [4H, D_IN]
    W_rv = W_ih[:, D_IN:]         # [4H, R*W]
    bias = b_ih + b_hh            # [4H]
    bias1 = bias + rv0.reshape(R * W) @ W_rv.T

    # G[128k + j, c] = (mem_sm.T @ W_rv[:, kM:(k+1)M].T)[j, c]
    G = np.concatenate(
        [mem_sm.T @ W_rv[:, k * M:(k + 1) * M].T for k in range(R)], axis=0
    )  # [512, 4H]

    wxT_h = np.ascontiguousarray(
        W_x.T.reshape(2, 128, 4 * H).transpose(1, 0, 2), dtype=np.float32
    )
    whhT_h = np.ascontiguousarray(W_hh.T, dtype=np.float32)
    gmat_h = np.ascontiguousarray(
        G.reshape(4, 128, 4 * H).transpose(1, 0, 2).astype(ml_dtypes.bfloat16)
    )
    biasc_h = np.ascontiguousarray(bias.astype(np.float32).reshape(4, 128).T)
    bias1c_h = np.ascontiguousarray(bias1.astype(np.float32).reshape(4, 128).T)

    in_maps = []
    for d in range(NCORES):
        xs = x[d * RL:(d + 1) * RL, :t_steps, :]          # [RL, t, 256]
        x2 = xs.transpose(1, 2, 0)                        # [t, 256, RL]
        xT_h = np.ascontiguousarray(
            x2.reshape(min(t_steps, T), 2, 128, RL).transpose(0, 2, 1, 3)
        )                                                 # [t, 128, 2, RL]
        in_maps.append(
            {
                "xT": xT_h,
                "wxT": wxT_h,
                "whhT": whhT_h,
                "gmat": gmat_h,
                "biasc": biasc_h,
                "bias1c": bias1c_h,
            }
        )
    return in_maps, mem_sm


def host_finish(inputs, hsum, t_steps=T):
    """Final fc layer + time-mean from hsum [B, H] (linear in hsum)."""
    memory = np.asarray(inputs["memory"], dtype=np.float64)
    fc_w = np.asarray(inputs["fc_w"], dtype=np.float64)
    fc_b = np.asarray(inputs["fc_b"], dtype=np.float64)

    mm = memory - memory.max(axis=0, keepdims=True)
    e = np.exp(mm)
    mem_sm = e / e.sum(axis=0, keepdims=True)

    fc_h = fc_w[:, :H]  # [OUT, H]
    Fstack = np.concatenate(
        [mem_sm.T @ fc_w[:, H + k * M:H + (k + 1) * M].T for k in range(R)],
        axis=0,
    )  # [512, OUT]

    hs = hsum.astype(np.float64)
    mixout = hs.reshape(B // 4, 4 * H) @ Fstack           # [256, OUT]
    out = (hs @ fc_h.T + mixout[np.arange(B) % (B // 4)]) / t_steps + fc_b
    return out.astype(np.float32)


def kernel(**inputs):
    """Entry point: full inputs in, full [B, OUT] output back."""
    from concourse.bass_utils import run_bass_kernel_spmd

    key = ("v3", T)
    if key not in _PROGRAM_CACHE:
        _PROGRAM_CACHE[key] = build_program_v3(T)
    nc = _PROGRAM_CACHE[key]

    in_maps, _ = host_prep(inputs, T, mode="v3")
    res = run_bass_kernel_spmd(nc, in_maps, core_ids=list(range(NCORES)))
    hsum = np.empty((B, H), np.float32)
    for d in range(NCORES):
        hsum[d * RL:(d + 1) * RL, :] = res.results[d]["hsum_out"].T
    return host_finish(inputs, hsum, T)


# revision 3
# speedup vs baseline: 1.5823x; 1.5823x over previous
"""DNC-style LSTM-with-memory-read kernel for 8 Trainium2 NeuronCores.

Math summary (derived from the reference):
  The torch-faithful [R,B,M]->[B,R*M] view means row b' of the new read
  vector is concat_k read[(4*b'+k) mod B]. Since read = h @ mem_sm.T and
  rv only enters the LSTM through W_ih's rv columns (W_rv), the rv
  contribution to the gates collapses to a "mix" term:
      gates[b'] += sum_k h[(4*u(b')+k) mod B] @ G_k,   u(b') = b' mod 256
  with G_k = mem_sm.T @ W_rv[:, k*M:(k+1)*M].T precomputed on host.
  The final fc layer is linear in h and read, and the output is a mean
  over time, so it reduces to a function of hsum = sum_t h_t — computed
  on host from each core's hsum shard.

Distribution (r3): per-step collectives cost ~80us on this fabric
(measured), so the recurrence is fully replicated: every core runs the
whole batch, zero collectives (x is replicated from the host). The step
is engine-optimized instead:

  - Transposed layout [units, batch] everywhere: h is produced directly
    as the lhsT-ready rhs for the next step's W_hh/mix matmuls.
  - The batch is stored in base-4 digit-REVERSED order. Under b -> 4u+k
    the digit-reversed index map turns the mix gather into contiguous
    slices: out col p needs h col 256k + p//4, so the mix matmuls read
    h[:, 256k+128h : +128] with a stride-0 inner broadcast (x4) rhs and
    accumulate straight into the gate PSUM banks — no deinterleave copy,
    no broadcast adds, no PSUM->SBUF mix copies.
  - All matmul inputs bf16 (2x PE throughput); PSUM accumulates fp32;
    the cell state c stays fp32.
  - ACT applies sigmoid/tanh directly from PSUM with the per-unit bias
    fused, writing SBUF.
"""

import sys

if '/opt/trn_rl_repo' not in sys.path:
    sys.path.insert(0, '/opt/trn_rl_repo')

import numpy as np

B, T, D_IN = 1024, 128, 256
H = 128
M = 128
W = 128
R = 4
OUT = 2
NCORES = 8
RL = B // NCORES

_PROGRAM_CACHE = {}


def _rev_perm():
    """base-4 digit reversal of 0..1023 (involution)."""
    b = np.arange(B)
    d = [(b >> (2 * i)) & 3 for i in range(5)]  # d0..d4, d0 least sig
    return d[0] * 256 + d[1] * 64 + d[2] * 16 + d[3] * 4 + d[4]


REV = _rev_perm()


def build_program_r3(t_steps=T):
    import concourse.bacc as bacc
    import concourse.mybir as mybir
    import concourse.tile as tile

    f32 = mybir.dt.float32
    bf16 = mybir.dt.bfloat16
    AF = mybir.ActivationFunctionType
    x_t_in = min(t_steps, T)

    nc = bacc.Bacc(
        "TRN2",
        target_bir_lowering=False,
        debug=False,
        enable_asserts=False,
        num_devices=NCORES,
    )

    xT = nc.dram_tensor("xT", [x_t_in, 128, 2, B], bf16, kind="ExternalInput")
    wxT = nc.dram_tensor("wxT", [128, 2, 512], bf16, kind="ExternalInput")
    whhT = nc.dram_tensor("whhT", [128, 512], bf16, kind="ExternalInput")
    gmat = nc.dram_tensor("gmat", [128, 4, 512], bf16, kind="ExternalInput")
    biasc = nc.dram_tensor("biasc", [128, 4], f32, kind="ExternalInput")
    bias1c = nc.dram_tensor("bias1c", [128, 4], f32, kind="ExternalInput")
    hsum_out = nc.dram_tensor("hsum_out", [128, B], f32, kind="ExternalOutput")

    FN = {0: AF.Sigmoid, 1: AF.Sigmoid, 2: AF.Tanh, 3: AF.Sigmoid}
    ORDER = (0, 2, 1, 3)  # i, g, f, o: t2=i*g first, o last

    with tile.TileContext(nc) as tc:
        with (
            tc.tile_pool(name="const", bufs=1) as cpool,
            tc.tile_pool(name="xin", bufs=3) as xpool,
            tc.tile_pool(name="work", bufs=2) as wpool,
            tc.tile_pool(name="psg", bufs=8, space="PSUM") as psg,
        ):
            wx_sb = cpool.tile([128, 2, 512], bf16)
            nc.sync.dma_start(wx_sb[:], wxT[:])
            whh_sb = cpool.tile([128, 512], bf16)
            nc.sync.dma_start(whh_sb[:], whhT[:])
            g_sb = cpool.tile([128, 4, 512], bf16)
            nc.sync.dma_start(g_sb[:], gmat[:])
            bb_sb = cpool.tile([128, 4], f32)
            nc.sync.dma_start(bb_sb[:], biasc[:])
            b1_sb = cpool.tile([128, 4], f32)
            nc.sync.dma_start(b1_sb[:], bias1c[:])
            hsum = cpool.tile([128, B], f32)
            nc.vector.memset(hsum[:], 0.0)

            h16 = None
            c_half = [None, None]

            for t in range(1, t_steps + 1):
                xt = xpool.tile([128, 2, B], bf16, tag="xt")
                nc.sync.dma_start(xt[:], xT[(t - 1) % x_t_in])

                # --- gate pre-activations: 8 PSUM banks (2 halves x 4
                # gates). x-projection first (independent of h), then
                # W_hh, then mix — so PE only stalls on h at the W_hh/mix
                # stage while next-step xproj fills the queue behind it.
                pg = {}
                for h_ in range(2):
                    rs = slice(512 * h_, 512 * (h_ + 1))
                    for g in ORDER:
                        gsl = slice(128 * g, 128 * (g + 1))
                        p = psg.tile([128, 512], f32, tag="pg",
                                     name=f"pg{h_}{g}")
                        pg[(h_, g)] = p
                        nc.tensor.matmul(
                            p[:], wx_sb[:, 0, gsl], xt[:, 0, rs],
                            start=True, stop=False,
                        )
                        nc.tensor.matmul(
                            p[:], wx_sb[:, 1, gsl], xt[:, 1, rs],
                            start=False, stop=(t == 1),
                        )
                if t >= 2:
                    for h_ in range(2):
                        rs = slice(512 * h_, 512 * (h_ + 1))
                        for g in ORDER:
                            gsl = slice(128 * g, 128 * (g + 1))
                            nc.tensor.matmul(
                                pg[(h_, g)][:], whh_sb[:, gsl], h16[:, rs],
                                start=False, stop=False,
                            )
                        for g in ORDER:
                            gsl = slice(128 * g, 128 * (g + 1))
                            for k in range(4):
                                lo = 256 * k + 128 * h_
                                rhs = (
                                    h16[:, lo:lo + 128]
                                    .unsqueeze(2)
                                    .broadcast_to([128, 128, 4])
                                )
                                nc.tensor.matmul(
                                    pg[(h_, g)][:], g_sb[:, k, gsl], rhs,
                                    start=False, stop=(k == 3),
                                )

                bias_t = b1_sb if t == 1 else bb_sb
                h16_new = wpool.tile([128, B], bf16, tag="h")
                for h_ in range(2):
                    rs = slice(512 * h_, 512 * (h_ + 1))
                    a = {}
                    for g in ORDER:
                        a[g] = wpool.tile([128, 512], f32, tag=f"a{h_}{g}",
                                          name=f"a{h_}{g}")
                        nc.scalar.activation(
                            a[g][:], pg[(h_, g)][:], FN[g],
                            bias=bias_t[:, g:g + 1],
                        )
                    t2 = wpool.tile([128, 512], f32, tag=f"t2{h_}")
                    nc.vector.tensor_mul(t2[:], a[0][:], a[2][:])
                    c_new = wpool.tile([128, 512], f32, tag=f"c{h_}")
                    if t == 1:
                        nc.vector.tensor_copy(c_new[:], t2[:])
                    else:
                        t1 = wpool.tile([128, 512], f32, tag=f"t1{h_}")
                        nc.vector.tensor_mul(t1[:], a[1][:], c_half[h_][:])
                        nc.vector.tensor_add(c_new[:], t1[:], t2[:])
                    c_half[h_] = c_new
                    tch = wpool.tile([128, 512], f32, tag=f"tch{h_}")
                    nc.scalar.activation(tch[:], c_new[:], AF.Tanh)
                    nc.vector.tensor_mul(h16_new[:, rs], a[3][:], tch[:])
                    nc.vector.tensor_add(hsum[:, rs], hsum[:, rs],
                                         h16_new[:, rs])
                h16 = h16_new

            nc.sync.dma_start(hsum_out[:], hsum[:])

    nc.compile()
    return nc


def host_prep(inputs, t_steps=T, mode="r3"):
    """Host-side parameter folding + per-core input maps."""
    import ml_dtypes

    x = np.asarray(inputs["x"], dtype=np.float32)
    memory = np.asarray(inputs["memory"], dtype=np.float64)
    rv0 = np.asarray(inputs["read_vectors0"], dtype=np.float64)
    W_ih = np.asarray(inputs["W_ih"], dtype=np.float64)
    W_hh = np.asarray(inputs["W_hh"], dtype=np.float64)
    b_ih = np.asarray(inputs["b_ih"], dtype=np.float64)
    b_hh = np.asarray(inputs["b_hh"], dtype=np.float64)

    mm = memory - memory.max(axis=0, keepdims=True)
    e = np.exp(mm)
    mem_sm = e / e.sum(axis=0, keepdims=True)  # [M, W]

    W_x = W_ih[:, :D_IN]          # [4H, D_IN]
    W_rv = W_ih[:, D_IN:]         # [4H, R*W]
    bias = b_ih + b_hh            # [4H]
    bias1 = bias + rv0.reshape(R * W) @ W_rv.T

    G = np.concatenate(
        [mem_sm.T @ W_rv[:, k * M:(k + 1) * M].T for k in range(R)], axis=0
    )  # [512, 4H]

    bf = ml_dtypes.bfloat16
    wxT_h = np.ascontiguousarray(
        W_x.T.reshape(2, 128, 4 * H).transpose(1, 0, 2).astype(bf)
    )
    whhT_h = np.ascontiguousarray(W_hh.T.astype(bf))
    gmat_h = np.ascontiguousarray(
        G.reshape(4, 128, 4 * H).transpose(1, 0, 2).astype(bf)
    )
    biasc_h = np.ascontiguousarray(bias.astype(np.float32).reshape(4, 128).T)
    bias1c_h = np.ascontiguousarray(bias1.astype(np.float32).reshape(4, 128).T)

    # full batch, digit-reversed order, transposed: [t, 128, 2, B] bf16
    ts_ = min(t_steps, T)
    xr = x[REV, :ts_, :]                                  # [B, t, 256]
    xT_h = np.ascontiguousarray(
        xr.transpose(1, 2, 0).reshape(ts_, 2, 128, B).transpose(0, 2, 1, 3)
        .astype(bf)
    )                                                     # [t, 128, 2, B]

    core_map = {
        "xT": xT_h,
        "wxT": wxT_h,
        "whhT": whhT_h,
        "gmat": gmat_h,
        "biasc": biasc_h,
        "bias1c": bias1c_h,
    }
    in_maps = [core_map for _ in range(NCORES)]
    return in_maps, mem_sm


def host_finish(inputs, hsum, t_steps=T):
    """Final fc layer + time-mean from hsum [B, H] (linear in hsum)."""
    memory = np.asarray(inputs["memory"], dtype=np.float64)
    fc_w = np.asarray(inputs["fc_w"], dtype=np.float64)
    fc_b = np.asarray(inputs["fc_b"], dtype=np.float64)

    mm = memory - memory.max(axis=0, keepdims=True)
    e = np.exp(mm)
    mem_sm = e / e.sum(axis=0, keepdims=True)

    fc_h = fc_w[:, :H]  # [OUT, H]
    Fstack = np.concatenate(
        [mem_sm.T @ fc_w[:, H + k * M:H + (k + 1) * M].T for k in range(R)],
        axis=0,
    )  # [512, OUT]

    hs = hsum.astype(np.float64)
    mixout = hs.reshape(B // 4, 4 * H) @ Fstack           # [256, OUT]
    out = (hs @ fc_h.T + mixout[np.arange(B) % (B // 4)]) / t_steps + fc_b
    return out.astype(np.float32)


def kernel(**inputs):
    """Entry point: full inputs in, full [B, OUT] output back."""
    from concourse.bass_utils import run_bass_kernel_spmd

    key = ("r3", T)
    if key not in _PROGRAM_CACHE:
        _PROGRAM_CACHE[key] = build_program_r3(T)
    nc = _PROGRAM_CACHE[key]

    in_maps, _ = host_prep(inputs, T)
    res = run_bass_kernel_spmd(nc, in_maps, core_ids=list(range(NCORES)))
    # [128, B] in digit-reversed batch order (all cores identical)
    hsumT = res.results[0]["hsum_out"]
    hsum = hsumT.T[REV]  # rev is an involution: row b = hsumT.T[rev(b)]
    return host_finish(inputs, hsum, T)
